# revision 6
# baseline (speedup 1.0000x reference)
"""AttentionOnAttention Trainium2 kernel (8 NeuronCores, SPMD).

Sharding: core c handles batch b = c//4 and heads [4*(c%4), 4*(c%4)+4).
Each core computes its disjoint output slice out[b, :, 256*(c%4):256*(c%4+1)];
no collectives are needed.

Per-core dataflow (everything in "transposed" orientation, partition = feature):
  xT (1024, 2048) --PE--> qT_h, kT_h (64, 2048) per head, v (2048, 256) natural
  S^T[j,i] = k_h qT_h          (K=64 contraction, PSUM [128j, 1024i] tiles)
  expS^T = exp(S^T * 1/8)      (ScalarE, scale fused into ACT)
  [ao^T; L] = [v|1]^T expS^T   (K=128j accumulation, ones column gives the
                                softmax denominator for free)
  ao^T *= 1/L                  (DVE recip + GPSIMD partition broadcast + DVE mul)
  [I^T; G^T] = [Wq_out^T|Wq_gate^T]^T qT + [Wattn_out^T|Wattn_gate^T]^T ao^T
  out^T = (I^T + b) * 1/(1 + exp(-(G^T + bg)))   (sigmoid via exp: no ACT
                                                  table-set switch)
Host transposes the per-core (256, 2048) outputs back.
"""

import numpy as np
from contextlib import ExitStack

import concourse.bass as bass
import concourse.bacc as bacc
import concourse.tile as tile
from concourse import mybir

B, N, DIM, H, DH = 2, 2048, 1024, 16, 64
HPC = H // 4          # 4 heads per core
INC = HPC * DH        # 256 per-core inner width
KT = DIM // 128       # 8 contraction tiles
NCH = N // 512        # 4 free-dim chunks of 512
SCALE = float(DH) ** -0.5
F32 = mybir.dt.float32
F32R = mybir.dt.float32r
AF = mybir.ActivationFunctionType
ALU = mybir.AluOpType

IH = 2                # i-halves per head
IHW = N // IH         # 1024 wide i-half
JT = N // 128         # 16 j tiles


def build_nc():
    nc = bacc.Bacc(
        "TRN2",
        target_bir_lowering=False,
        debug=False,
        enable_asserts=False,
        num_devices=8,
    )
    xT_d = nc.dram_tensor("xT", (DIM, N), F32R, kind="ExternalInput").ap()
    wq_d = nc.dram_tensor("wq", (DIM, INC), F32R, kind="ExternalInput").ap()
    wk_d = nc.dram_tensor("wk", (DIM, INC), F32R, kind="ExternalInput").ap()
    wv_d = nc.dram_tensor("wv", (DIM, INC), F32R, kind="ExternalInput").ap()
    wcq_d = nc.dram_tensor("wcq", (DH, 2 * DH), F32R, kind="ExternalInput").ap()
    wca_d = nc.dram_tensor("wca", (DH, 2 * DH), F32R, kind="ExternalInput").ap()
    bias_d = nc.dram_tensor("biases", (2 * DH, 1), F32, kind="ExternalInput").ap()
    ones_d = nc.dram_tensor("ones", (128, 1), F32R, kind="ExternalInput").ap()
    outT_d = nc.dram_tensor("outT", (INC, N), F32, kind="ExternalOutput").ap()

    with tile.TileContext(nc) as tc, ExitStack() as ctx:
        consts = ctx.enter_context(tc.tile_pool(name="consts", bufs=1))

        wcq_sb = consts.tile([DH, 2 * DH], F32R, name="wcq_sb")
        nc.sync.dma_start(out=wcq_sb, in_=wcq_d)
        wca_sb = consts.tile([DH, 2 * DH], F32R, name="wca_sb")
        nc.sync.dma_start(out=wca_sb, in_=wca_d)
        bias_sb = consts.tile([2 * DH, 1], F32, name="bias_sb")
        nc.sync.dma_start(out=bias_sb, in_=bias_d)

        # persistent per-head tensors
        qT = [consts.tile([DH, N], F32R, name=f"qT{h}") for h in range(HPC)]
        kT = [consts.tile([DH, N], F32R, name=f"kT{h}") for h in range(HPC)]
        v_aug = consts.tile([128, JT, HPC, DH + 1], F32R, name="v_aug")
        ones_sb = consts.tile([128, 1], F32R, name="ones_sb")
        nc.sync.dma_start(out=ones_sb, in_=ones_d)
        ones_bcast = bass.AP(
            tensor=ones_sb.tensor,
            offset=ones_sb.offset,
            ap=[list(ones_sb.ap[0]), [0, JT], [0, HPC], [0, 1]],
        )
        nc.vector.tensor_copy(out=v_aug[:, :, :, DH : DH + 1], in_=ones_bcast)

        # ---------------- projections ----------------
        with (
            tc.tile_pool(name="xw", bufs=1) as xw,
            tc.tile_pool(name="proj_ps", bufs=2, space="PSUM") as pps,
        ):
            wq_sb = xw.tile([128, KT, INC], F32R, name="wq_sb")
            wk_sb = xw.tile([128, KT, INC], F32R, name="wk_sb")
            wv_sb = xw.tile([128, KT, INC], F32R, name="wv_sb")
            for k in range(KT):
                ks = slice(k * 128, (k + 1) * 128)
                nc.sync.dma_start(out=wq_sb[:, k, :], in_=wq_d[ks, :])
                nc.sync.dma_start(out=wk_sb[:, k, :], in_=wk_d[ks, :])
                nc.sync.dma_start(out=wv_sb[:, k, :], in_=wv_d[ks, :])

            xt_sb = xw.tile([128, KT, N], F32R, name="xt_sb")
            for c in range(NCH):
                cs = slice(c * 512, (c + 1) * 512)
                for k in range(KT):
                    nc.sync.dma_start(
                        out=xt_sb[:, k, cs], in_=xT_d[k * 128 : (k + 1) * 128, cs]
                    )

            for c in range(NCH):
                cs = slice(c * 512, (c + 1) * 512)
                for m in range(2):  # inner m-tile: heads 2m, 2m+1
                    for wsb, dst in ((wq_sb, qT), (wk_sb, kT)):
                        ps = pps.tile([128, 512], F32, name="ps_qk", tag="ps_qk")
                        for k in range(KT):
                            nc.tensor.matmul(
                                ps,
                                lhsT=(wsb[:, k, m * 128 : (m + 1) * 128]),
                                rhs=(xt_sb[:, k, cs]),
                                start=(k == 0),
                                stop=(k == KT - 1),
                            )
                        nc.vector.tensor_copy(out=dst[2 * m][:, cs], in_=ps[0:DH, :])
                        nc.vector.tensor_copy(
                            out=dst[2 * m + 1][:, cs], in_=ps[DH:128, :]
                        )
                # v natural: i-tiles of this chunk
                for it in range(c * 4, c * 4 + 4):
                    psv = pps.tile([128, INC], F32, name="ps_v", tag="ps_v")
                    for k in range(KT):
                        nc.tensor.matmul(
                            psv,
                            lhsT=(xt_sb[:, k, it * 128 : (it + 1) * 128]),
                            rhs=(wv_sb[:, k, :]),
                            start=(k == 0),
                            stop=(k == KT - 1),
                        )
                    nc.vector.tensor_copy(
                        out=v_aug[:, it, :, 0:DH],
                        in_=psv.rearrange("p (h d) -> p h d", h=HPC),
                    )

        # ---------------- attention ----------------
        aoT = [consts.tile([DH, N], F32R, name=f"aoT{h}") for h in range(HPC)]
        with (
            tc.tile_pool(name="s_ps", bufs=2, space="PSUM") as sps,
            tc.tile_pool(name="pv_ps", bufs=2, space="PSUM") as pvps,
            tc.tile_pool(name="es_p", bufs=3) as esp,
            tc.tile_pool(name="norm_p", bufs=2) as nrm,
        ):
            for h in range(HPC):
                for ih in range(IH):
                    isl = slice(ih * IHW, (ih + 1) * IHW)
                    pv = pvps.tile([DH + 1, IHW], F32, name="pv", tag="pv")
                    es_tiles = [None] * JT

                    def emit_pv(jt):
                        for cc in range(IHW // 512):
                            nc.tensor.matmul(
                                pv[:, cc * 512 : (cc + 1) * 512],
                                lhsT=(v_aug[:, jt, h, :]),
                                rhs=(es_tiles[jt][:, cc * 512 : (cc + 1) * 512]),
                                start=(jt == 0),
                                stop=(jt == JT - 1),
                            )

                    for jt in range(JT):
                        s = sps.tile([128, IHW], F32, name="s", tag="s")
                        for cc in range(IHW // 512):
                            qs = slice(ih * IHW + cc * 512, ih * IHW + (cc + 1) * 512)
                            nc.tensor.matmul(
                                s[:, cc * 512 : (cc + 1) * 512],
                                lhsT=(kT[h][:, jt * 128 : (jt + 1) * 128]),
                                rhs=(qT[h][:, qs]),
                                start=True,
                                stop=True,
                            )
                        es = esp.tile([128, IHW], F32R, name="es", tag="es")
                        nc.scalar.activation(out=es, in_=s, func=AF.Exp, scale=SCALE)
                        es_tiles[jt] = es
                        # keep PE one S-tile ahead of the PV consumer
                        if jt > 0:
                            emit_pv(jt - 1)
                    emit_pv(JT - 1)

                    # normalize: aoT = pv[0:64] / pv[64]
                    rl = nrm.tile([1, IHW], F32, name="rl", tag="rl")
                    nc.vector.reciprocal(out=rl, in_=pv[DH : DH + 1, :])
                    rlb = nrm.tile([DH, IHW], F32, name="rlb", tag="rlb")
                    nc.gpsimd.partition_broadcast(rlb, rl)
                    nc.vector.tensor_mul(out=aoT[h][:, isl], in0=pv[0:DH, :], in1=rlb)

        # ---------------- AoA output + gate ----------------
        with (
            tc.tile_pool(name="ig_ps", bufs=2, space="PSUM") as igp,
            tc.tile_pool(name="fin_p", bufs=2) as fin,
        ):
            for h in range(HPC):
                ig = igp.tile([128, N], F32, name="ig", tag="ig")
                for c in range(NCH):
                    cs = slice(c * 512, (c + 1) * 512)
                    nc.tensor.matmul(
                        ig[:, cs],
                        lhsT=(wcq_sb),
                        rhs=(qT[h][:, cs]),
                        start=True,
                        stop=False,
                    )
                    nc.tensor.matmul(
                        ig[:, cs],
                        lhsT=(wca_sb),
                        rhs=(aoT[h][:, cs]),
                        start=False,
                        stop=True,
                    )
                # sigmoid(G + bg) = 1 / (1 + exp(-G - bg)); bias_sb[64:] = -bg
                eg = fin.tile([DH, N], F32, name="eg", tag="eg")
                nc.scalar.activation(
                    out=eg,
                    in_=ig[DH:128, :],
                    func=AF.Exp,
                    scale=-1.0,
                    bias=bias_sb[DH : 2 * DH, :],
                )
                nc.vector.tensor_scalar_add(eg, eg, 1.0)
                sg = fin.tile([DH, N], F32, name="sg", tag="sg")
                nc.vector.reciprocal(out=sg, in_=eg)
                ot = fin.tile([DH, N], F32, name="ot", tag="ot")
                nc.vector.scalar_tensor_tensor(
                    out=ot,
                    in0=ig[0:DH, :],
                    scalar=bias_sb[0:DH, :],
                    in1=sg,
                    op0=ALU.add,
                    op1=ALU.mult,
                )
                nc.sync.dma_start(out=outT_d[h * DH : (h + 1) * DH, :], in_=ot)

    nc.compile()
    return nc


_NC_CACHE = None


def _get_nc():
    global _NC_CACHE
    if _NC_CACHE is None:
        _NC_CACHE = build_nc()
    return _NC_CACHE


def make_in_maps(x, Wq, Wkv, Wq_out, Wattn_out, out_bias, Wq_gate, Wattn_gate,
                 gate_bias):
    wcq = np.ascontiguousarray(np.concatenate([Wq_out.T, Wq_gate.T], axis=1),
                               dtype=np.float32)
    wca = np.ascontiguousarray(
        np.concatenate([Wattn_out.T, Wattn_gate.T], axis=1), dtype=np.float32
    )
    biases = np.concatenate(
        [out_bias.reshape(-1), -gate_bias.reshape(-1)]
    ).astype(np.float32).reshape(2 * DH, 1)
    biases = np.ascontiguousarray(biases)
    Wk = Wkv[:, : H * DH]
    Wv = Wkv[:, H * DH :]
    xT = [np.ascontiguousarray(x[b].T, dtype=np.float32) for b in range(B)]
    in_maps = []
    for c in range(8):
        b, hg = c // 4, c % 4
        cols = slice(hg * INC, (hg + 1) * INC)
        in_maps.append(
            {
                "xT": xT[b],
                "wq": np.ascontiguousarray(Wq[:, cols], dtype=np.float32),
                "wk": np.ascontiguousarray(Wk[:, cols], dtype=np.float32),
                "wv": np.ascontiguousarray(Wv[:, cols], dtype=np.float32),
                "wcq": wcq,
                "wca": wca,
                "biases": biases,
                "ones": np.ones((128, 1), dtype=np.float32),
            }
        )
    return in_maps


def assemble_output(results):
    out = np.empty((B, N, H * DH), dtype=np.float32)
    for c in range(8):
        b, hg = c // 4, c % 4
        out[b, :, hg * INC : (hg + 1) * INC] = results[c]["outT"].T
    return out


def kernel(**inputs):
    from concourse.bass_utils import run_bass_kernel_spmd

    inputs = {k: np.asarray(v, dtype=np.float32) for k, v in inputs.items()}
    nc = _get_nc()
    in_maps = make_in_maps(**inputs)
    res = run_bass_kernel_spmd(nc, in_maps, core_ids=list(range(8)))
    return assemble_output(res.results)


# revision 13
# speedup vs baseline: 1.5660x; 1.5660x over previous
"""AttentionOnAttention Trainium2 kernel (8 NeuronCores, SPMD).

Sharding: core c handles batch b = c//4 and heads [4*(c%4), 4*(c%4)+4).
Each core computes its disjoint output slice out[b, :, 256*(c%4):256*(c%4+1)];
no collectives are needed.

Per-core dataflow (everything in "transposed" orientation, partition = feature):
  xT (1024, 2048) --PE--> qT_h, kT_h (64, 2048) per head, v (2048, 256) natural
  S^T[j,i] = k_h qT_h          (K=64 contraction, PSUM [128j, 1024i] tiles)
  expS^T = exp(S^T * 1/8)      (ScalarE, scale fused into ACT)
  [ao^T; L] = [v|1]^T expS^T   (K=128j accumulation, ones column gives the
                                softmax denominator for free)
  ao^T *= 1/L                  (DVE recip + GPSIMD partition broadcast + DVE mul)
  [I^T; G^T] = [Wq_out^T|Wq_gate^T]^T qT + [Wattn_out^T|Wattn_gate^T]^T ao^T
  out^T = (I^T + b) * 1/(1 + exp(-(G^T + bg)))   (sigmoid via exp: no ACT
                                                  table-set switch)
Host transposes the per-core (256, 2048) outputs back.
"""

import numpy as np
from contextlib import ExitStack

import concourse.bass as bass
import concourse.bacc as bacc
import concourse.tile as tile
from concourse import mybir

B, N, DIM, H, DH = 2, 2048, 1024, 16, 64
HPC = H // 4          # 4 heads per core
INC = HPC * DH        # 256 per-core inner width
KT = DIM // 128       # 8 contraction tiles
NCH = N // 512        # 4 free-dim chunks of 512
SCALE = float(DH) ** -0.5
F32 = mybir.dt.float32
F32R = mybir.dt.float32r
AF = mybir.ActivationFunctionType
ALU = mybir.AluOpType

IH = 2                # i-halves per head
IHW = N // IH         # 1024 wide i-half
JT = N // 128         # 16 j tiles


def build_nc():
    nc = bacc.Bacc(
        "TRN2",
        target_bir_lowering=False,
        debug=False,
        enable_asserts=False,
        num_devices=8,
    )
    xT_d = nc.dram_tensor("xT", (DIM, N), F32R, kind="ExternalInput").ap()
    wq_d = nc.dram_tensor("wq", (DIM, INC), F32R, kind="ExternalInput").ap()
    wk_d = nc.dram_tensor("wk", (DIM, INC), F32R, kind="ExternalInput").ap()
    wv_d = nc.dram_tensor("wv", (DIM, INC), F32R, kind="ExternalInput").ap()
    wcq_d = nc.dram_tensor("wcq", (DH, 2 * DH), F32R, kind="ExternalInput").ap()
    wca_d = nc.dram_tensor("wca", (DH, 2 * DH), F32R, kind="ExternalInput").ap()
    bias_d = nc.dram_tensor("biases", (2 * DH, 1), F32, kind="ExternalInput").ap()
    ones_d = nc.dram_tensor("ones", (128, 1), F32R, kind="ExternalInput").ap()
    outT_d = nc.dram_tensor("outT", (INC, N), F32, kind="ExternalOutput").ap()

    with tile.TileContext(nc) as tc, ExitStack() as ctx:
        consts = ctx.enter_context(tc.tile_pool(name="consts", bufs=1))

        wcq_sb = consts.tile([DH, 2 * DH], F32R, name="wcq_sb")
        nc.sync.dma_start(out=wcq_sb, in_=wcq_d)
        wca_sb = consts.tile([DH, 2 * DH], F32R, name="wca_sb")
        nc.sync.dma_start(out=wca_sb, in_=wca_d)
        bias_sb = consts.tile([2 * DH, 1], F32, name="bias_sb")
        nc.sync.dma_start(out=bias_sb, in_=bias_d)

        # persistent per-head tensors
        qT = [consts.tile([DH, N], F32R, name=f"qT{h}") for h in range(HPC)]
        kT = [consts.tile([DH, N], F32R, name=f"kT{h}") for h in range(HPC)]
        v_aug = consts.tile([128, JT, HPC, DH + 1], F32R, name="v_aug")
        ones_sb = consts.tile([128, 1], F32R, name="ones_sb")
        nc.sync.dma_start(out=ones_sb, in_=ones_d)
        ones_bcast = bass.AP(
            tensor=ones_sb.tensor,
            offset=ones_sb.offset,
            ap=[list(ones_sb.ap[0]), [0, JT], [0, HPC], [0, 1]],
        )
        nc.vector.tensor_copy(out=v_aug[:, :, :, DH : DH + 1], in_=ones_bcast)

        # Prefetch the exp/tanh ACT table set during the DMA prologue so the
        # first attention exp doesn't stall PE long enough to re-throttle HAM.
        warm_sb = consts.tile([128, 1], F32, name="warm_sb")
        nc.scalar.activation(out=warm_sb, in_=ones_sb.bitcast(F32), func=AF.Exp)
        nc.scalar.activation(out=warm_sb, in_=warm_sb, func=AF.Tanh)

        # ---------------- projections ----------------
        with (
            tc.tile_pool(name="xw", bufs=1) as xw,
            tc.tile_pool(name="proj_ps", bufs=2, space="PSUM") as pps,
        ):
            wq_sb = xw.tile([128, KT, INC], F32R, name="wq_sb")
            wk_sb = xw.tile([128, KT, INC], F32R, name="wk_sb")
            wv_sb = xw.tile([128, KT, INC], F32R, name="wv_sb")
            for k in range(KT):
                ks = slice(k * 128, (k + 1) * 128)
                nc.sync.dma_start(out=wq_sb[:, k, :], in_=wq_d[ks, :])
                nc.sync.dma_start(out=wk_sb[:, k, :], in_=wk_d[ks, :])
                nc.sync.dma_start(out=wv_sb[:, k, :], in_=wv_d[ks, :])

            xt_sb = xw.tile([128, KT, N], F32R, name="xt_sb")
            for c in range(NCH):
                cs = slice(c * 512, (c + 1) * 512)
                for k in range(KT):
                    nc.sync.dma_start(
                        out=xt_sb[:, k, cs], in_=xT_d[k * 128 : (k + 1) * 128, cs]
                    )

            for c in range(NCH):
                cs = slice(c * 512, (c + 1) * 512)
                for m in range(2):  # inner m-tile: heads 2m, 2m+1
                    for wsb, dst in ((wq_sb, qT), (wk_sb, kT)):
                        ps = pps.tile([128, 512], F32, name="ps_qk", tag="ps_qk")
                        for k in range(KT):
                            nc.tensor.matmul(
                                ps,
                                lhsT=(wsb[:, k, m * 128 : (m + 1) * 128]),
                                rhs=(xt_sb[:, k, cs]),
                                start=(k == 0),
                                stop=(k == KT - 1),
                            )
                        nc.vector.tensor_copy(out=dst[2 * m][:, cs], in_=ps[0:DH, :])
                        nc.vector.tensor_copy(
                            out=dst[2 * m + 1][:, cs], in_=ps[DH:128, :]
                        )
                # v natural: i-tiles of this chunk
                for it in range(c * 4, c * 4 + 4):
                    psv = pps.tile([128, INC], F32, name="ps_v", tag="ps_v")
                    for k in range(KT):
                        nc.tensor.matmul(
                            psv,
                            lhsT=(xt_sb[:, k, it * 128 : (it + 1) * 128]),
                            rhs=(wv_sb[:, k, :]),
                            start=(k == 0),
                            stop=(k == KT - 1),
                        )
                    nc.vector.tensor_copy(
                        out=v_aug[:, it, :, 0:DH],
                        in_=psv.rearrange("p (h d) -> p h d", h=HPC),
                    )

        # ---------------- attention ----------------
        aoT = [consts.tile([DH, N], F32R, name=f"aoT{h}") for h in range(HPC)]
        with (
            tc.tile_pool(name="s_ps", bufs=2, space="PSUM") as sps,
            tc.tile_pool(name="pv_ps", bufs=2, space="PSUM") as pvps,
            tc.tile_pool(name="es_p", bufs=3) as esp,
            tc.tile_pool(name="norm_p", bufs=2) as nrm,
        ):
            for h in range(HPC):
                for ih in range(IH):
                    isl = slice(ih * IHW, (ih + 1) * IHW)
                    pv = pvps.tile([DH + 1, IHW], F32, name="pv", tag="pv")
                    es_tiles = [None] * JT

                    def emit_pv(jt):
                        for cc in range(IHW // 512):
                            nc.tensor.matmul(
                                pv[:, cc * 512 : (cc + 1) * 512],
                                lhsT=(v_aug[:, jt, h, :]),
                                rhs=(es_tiles[jt][:, cc * 512 : (cc + 1) * 512]),
                                start=(jt == 0),
                                stop=(jt == JT - 1),
                            )

                    for jt in range(JT):
                        s = sps.tile([128, IHW], F32, name="s", tag="s")
                        for cc in range(IHW // 512):
                            qs = slice(ih * IHW + cc * 512, ih * IHW + (cc + 1) * 512)
                            nc.tensor.matmul(
                                s[:, cc * 512 : (cc + 1) * 512],
                                lhsT=(kT[h][:, jt * 128 : (jt + 1) * 128]),
                                rhs=(qT[h][:, qs]),
                                start=True,
                                stop=True,
                            )
                        es = esp.tile([128, IHW], F32R, name="es", tag="es")
                        nc.scalar.activation(out=es, in_=s, func=AF.Exp, scale=SCALE)
                        es_tiles[jt] = es
                        # keep PE one S-tile ahead of the PV consumer
                        if jt > 0:
                            emit_pv(jt - 1)
                    emit_pv(JT - 1)

                    # normalize: aoT = pv[0:64] / pv[64]
                    rl = nrm.tile([1, IHW], F32, name="rl", tag="rl")
                    nc.vector.reciprocal(out=rl, in_=pv[DH : DH + 1, :])
                    rlb = nrm.tile([DH, IHW], F32, name="rlb", tag="rlb")
                    nc.gpsimd.partition_broadcast(rlb, rl)
                    nc.vector.tensor_mul(out=aoT[h][:, isl], in0=pv[0:DH, :], in1=rlb)

        # ---------------- AoA output + gate ----------------
        with (
            tc.tile_pool(name="ig_ps", bufs=2, space="PSUM") as igp,
            tc.tile_pool(name="fin_p", bufs=2) as fin,
        ):
            for h in range(HPC):
                ig = igp.tile([128, N], F32, name="ig", tag="ig")
                for c in range(NCH):
                    cs = slice(c * 512, (c + 1) * 512)
                    nc.tensor.matmul(
                        ig[:, cs],
                        lhsT=(wcq_sb),
                        rhs=(qT[h][:, cs]),
                        start=True,
                        stop=False,
                    )
                    nc.tensor.matmul(
                        ig[:, cs],
                        lhsT=(wca_sb),
                        rhs=(aoT[h][:, cs]),
                        start=False,
                        stop=True,
                    )
                # sigmoid(G + bg) = 0.5 + 0.5*tanh((G + bg)/2); tanh shares the
                # exp ACT table set so no table switch. bias_sb[64:] = bg/2.
                tg = fin.tile([DH, N], F32, name="tg", tag="tg")
                nc.scalar.activation(
                    out=tg,
                    in_=ig[DH:128, :],
                    func=AF.Tanh,
                    scale=0.5,
                    bias=bias_sb[DH : 2 * DH, :],
                )
                sg = fin.tile([DH, N], F32, name="sg", tag="sg")
                nc.vector.tensor_scalar(
                    out=sg,
                    in0=tg,
                    scalar1=0.5,
                    scalar2=0.5,
                    op0=ALU.mult,
                    op1=ALU.add,
                )
                ot = fin.tile([DH, N], F32, name="ot", tag="ot")
                nc.vector.scalar_tensor_tensor(
                    out=ot,
                    in0=ig[0:DH, :],
                    scalar=bias_sb[0:DH, :],
                    in1=sg,
                    op0=ALU.add,
                    op1=ALU.mult,
                )
                nc.sync.dma_start(out=outT_d[h * DH : (h + 1) * DH, :], in_=ot)

    nc.compile()
    return nc


_NC_CACHE = None


def _get_nc():
    global _NC_CACHE
    if _NC_CACHE is None:
        _NC_CACHE = build_nc()
    return _NC_CACHE


def make_in_maps(x, Wq, Wkv, Wq_out, Wattn_out, out_bias, Wq_gate, Wattn_gate,
                 gate_bias):
    wcq = np.ascontiguousarray(np.concatenate([Wq_out.T, Wq_gate.T], axis=1),
                               dtype=np.float32)
    wca = np.ascontiguousarray(
        np.concatenate([Wattn_out.T, Wattn_gate.T], axis=1), dtype=np.float32
    )
    biases = np.concatenate(
        [out_bias.reshape(-1), 0.5 * gate_bias.reshape(-1)]
    ).astype(np.float32).reshape(2 * DH, 1)
    biases = np.ascontiguousarray(biases)
    Wk = Wkv[:, : H * DH]
    Wv = Wkv[:, H * DH :]
    xT = [np.ascontiguousarray(x[b].T, dtype=np.float32) for b in range(B)]
    in_maps = []
    for c in range(8):
        b, hg = c // 4, c % 4
        cols = slice(hg * INC, (hg + 1) * INC)
        in_maps.append(
            {
                "xT": xT[b],
                "wq": np.ascontiguousarray(Wq[:, cols], dtype=np.float32),
                "wk": np.ascontiguousarray(Wk[:, cols], dtype=np.float32),
                "wv": np.ascontiguousarray(Wv[:, cols], dtype=np.float32),
                "wcq": wcq,
                "wca": wca,
                "biases": biases,
                "ones": np.ones((128, 1), dtype=np.float32),
            }
        )
    return in_maps


def assemble_output(results):
    out = np.empty((B, N, H * DH), dtype=np.float32)
    for c in range(8):
        b, hg = c // 4, c % 4
        out[b, :, hg * INC : (hg + 1) * INC] = results[c]["outT"].T
    return out


def kernel(**inputs):
    from concourse.bass_utils import run_bass_kernel_spmd

    inputs = {k: np.asarray(v, dtype=np.float32) for k, v in inputs.items()}
    nc = _get_nc()
    in_maps = make_in_maps(**inputs)
    res = run_bass_kernel_spmd(nc, in_maps, core_ids=list(range(8)))
    return assemble_output(res.results)


# revision 14
# speedup vs baseline: 1.6902x; 1.0793x over previous
"""AttentionOnAttention Trainium2 kernel (8 NeuronCores, SPMD).

Sharding: core c handles batch b = c//4 and heads [4*(c%4), 4*(c%4)+4).
Each core computes its disjoint output slice out[b, :, 256*(c%4):256*(c%4+1)];
no collectives are needed.

Per-core dataflow (everything in "transposed" orientation, partition = feature):
  xT (1024, 2048) --PE--> qT_h, kT_h (64, 2048) per head, v (2048, 256) natural
  S^T[j,i] = k_h qT_h          (K=64 contraction, PSUM [128j, 1024i] tiles)
  expS^T = exp(S^T * 1/8)      (ScalarE, scale fused into ACT)
  [ao^T; L] = [v|1]^T expS^T   (K=128j accumulation, ones column gives the
                                softmax denominator for free)
  ao^T *= 1/L                  (DVE recip + GPSIMD partition broadcast + DVE mul)
  [I^T; G^T] = [Wq_out^T|Wq_gate^T]^T qT + [Wattn_out^T|Wattn_gate^T]^T ao^T
  out^T = (I^T + b) * 1/(1 + exp(-(G^T + bg)))   (sigmoid via exp: no ACT
                                                  table-set switch)
Host transposes the per-core (256, 2048) outputs back.
"""

import numpy as np
from contextlib import ExitStack

import concourse.bass as bass
import concourse.bacc as bacc
import concourse.tile as tile
from concourse import mybir

B, N, DIM, H, DH = 2, 2048, 1024, 16, 64
HPC = H // 4          # 4 heads per core
INC = HPC * DH        # 256 per-core inner width
KT = DIM // 128       # 8 contraction tiles
NCH = N // 512        # 4 free-dim chunks of 512
SCALE = float(DH) ** -0.5
F32 = mybir.dt.float32
F32R = mybir.dt.float32r
BF16 = mybir.dt.bfloat16
AF = mybir.ActivationFunctionType
ALU = mybir.AluOpType

IH = 2                # i-halves per head
IHW = N // IH         # 1024 wide i-half
JT = N // 128         # 16 j tiles


def build_nc():
    nc = bacc.Bacc(
        "TRN2",
        target_bir_lowering=False,
        debug=False,
        enable_asserts=False,
        num_devices=8,
    )
    xT_d = nc.dram_tensor("xT", (DIM, N), BF16, kind="ExternalInput").ap()
    wq_d = nc.dram_tensor("wq", (DIM, INC), BF16, kind="ExternalInput").ap()
    wk_d = nc.dram_tensor("wk", (DIM, INC), BF16, kind="ExternalInput").ap()
    wv_d = nc.dram_tensor("wv", (DIM, INC), BF16, kind="ExternalInput").ap()
    wcq_d = nc.dram_tensor("wcq", (DH, 2 * DH), BF16, kind="ExternalInput").ap()
    wca_d = nc.dram_tensor("wca", (DH, 2 * DH), BF16, kind="ExternalInput").ap()
    bias_d = nc.dram_tensor("biases", (2 * DH, 1), F32, kind="ExternalInput").ap()
    outT_d = nc.dram_tensor("outT", (INC, N), F32, kind="ExternalOutput").ap()

    with tile.TileContext(nc) as tc, ExitStack() as ctx:
        consts = ctx.enter_context(tc.tile_pool(name="consts", bufs=1))

        wcq_sb = consts.tile([DH, 2 * DH], BF16, name="wcq_sb")
        nc.sync.dma_start(out=wcq_sb, in_=wcq_d)
        wca_sb = consts.tile([DH, 2 * DH], BF16, name="wca_sb")
        nc.sync.dma_start(out=wca_sb, in_=wca_d)
        bias_sb = consts.tile([2 * DH, 1], F32, name="bias_sb")
        nc.sync.dma_start(out=bias_sb, in_=bias_d)

        # persistent per-head tensors
        qT = [consts.tile([DH, N], BF16, name=f"qT{h}") for h in range(HPC)]
        kT = [consts.tile([DH, N], BF16, name=f"kT{h}") for h in range(HPC)]
        v_aug = consts.tile([128, JT, HPC, DH + 1], BF16, name="v_aug")
        nc.vector.memset(v_aug[:, :, :, DH : DH + 1], 1.0)

        # Prefetch the exp/tanh ACT table set during the DMA prologue so the
        # first attention exp doesn't stall PE long enough to re-throttle HAM.
        warm_sb = consts.tile([128, 1], F32, name="warm_sb")
        nc.scalar.activation(out=warm_sb, in_=bias_sb, func=AF.Exp)
        nc.scalar.activation(out=warm_sb, in_=warm_sb, func=AF.Tanh)

        # ---------------- projections ----------------
        with (
            tc.tile_pool(name="xw", bufs=1) as xw,
            tc.tile_pool(name="proj_ps", bufs=2, space="PSUM") as pps,
        ):
            wq_sb = xw.tile([128, KT, INC], BF16, name="wq_sb")
            wk_sb = xw.tile([128, KT, INC], BF16, name="wk_sb")
            wv_sb = xw.tile([128, KT, INC], BF16, name="wv_sb")
            for k in range(KT):
                ks = slice(k * 128, (k + 1) * 128)
                nc.sync.dma_start(out=wq_sb[:, k, :], in_=wq_d[ks, :])
                nc.sync.dma_start(out=wk_sb[:, k, :], in_=wk_d[ks, :])
                nc.sync.dma_start(out=wv_sb[:, k, :], in_=wv_d[ks, :])

            xt_sb = xw.tile([128, KT, N], BF16, name="xt_sb")
            for c in range(NCH):
                cs = slice(c * 512, (c + 1) * 512)
                for k in range(KT):
                    nc.sync.dma_start(
                        out=xt_sb[:, k, cs], in_=xT_d[k * 128 : (k + 1) * 128, cs]
                    )

            for c in range(NCH):
                cs = slice(c * 512, (c + 1) * 512)
                for m in range(2):  # inner m-tile: heads 2m, 2m+1
                    for wsb, dst in ((wq_sb, qT), (wk_sb, kT)):
                        ps = pps.tile([128, 512], F32, name="ps_qk", tag="ps_qk")
                        for k in range(KT):
                            nc.tensor.matmul(
                                ps,
                                lhsT=(wsb[:, k, m * 128 : (m + 1) * 128]),
                                rhs=(xt_sb[:, k, cs]),
                                start=(k == 0),
                                stop=(k == KT - 1),
                            )
                        nc.vector.tensor_copy(out=dst[2 * m][:, cs], in_=ps[0:DH, :])
                        nc.vector.tensor_copy(
                            out=dst[2 * m + 1][:, cs], in_=ps[DH:128, :]
                        )
                # v natural: i-tiles of this chunk
                for it in range(c * 4, c * 4 + 4):
                    psv = pps.tile([128, INC], F32, name="ps_v", tag="ps_v")
                    for k in range(KT):
                        nc.tensor.matmul(
                            psv,
                            lhsT=(xt_sb[:, k, it * 128 : (it + 1) * 128]),
                            rhs=(wv_sb[:, k, :]),
                            start=(k == 0),
                            stop=(k == KT - 1),
                        )
                    nc.vector.tensor_copy(
                        out=v_aug[:, it, :, 0:DH],
                        in_=psv.rearrange("p (h d) -> p h d", h=HPC),
                    )

        # ---------------- attention ----------------
        aoT = [consts.tile([DH, N], BF16, name=f"aoT{h}") for h in range(HPC)]
        with (
            tc.tile_pool(name="s_ps", bufs=2, space="PSUM") as sps,
            tc.tile_pool(name="pv_ps", bufs=2, space="PSUM") as pvps,
            tc.tile_pool(name="es_p", bufs=3) as esp,
            tc.tile_pool(name="norm_p", bufs=2) as nrm,
        ):
            for h in range(HPC):
                for ih in range(IH):
                    isl = slice(ih * IHW, (ih + 1) * IHW)
                    pv = pvps.tile([DH + 1, IHW], F32, name="pv", tag="pv")
                    es_tiles = [None] * JT

                    def emit_pv(jt):
                        for cc in range(IHW // 512):
                            nc.tensor.matmul(
                                pv[:, cc * 512 : (cc + 1) * 512],
                                lhsT=(v_aug[:, jt, h, :]),
                                rhs=(es_tiles[jt][:, cc * 512 : (cc + 1) * 512]),
                                start=(jt == 0),
                                stop=(jt == JT - 1),
                            )

                    for jt in range(JT):
                        s = sps.tile([128, IHW], F32, name="s", tag="s")
                        for cc in range(IHW // 512):
                            qs = slice(ih * IHW + cc * 512, ih * IHW + (cc + 1) * 512)
                            nc.tensor.matmul(
                                s[:, cc * 512 : (cc + 1) * 512],
                                lhsT=(kT[h][:, jt * 128 : (jt + 1) * 128]),
                                rhs=(qT[h][:, qs]),
                                start=True,
                                stop=True,
                            )
                        es = esp.tile([128, IHW], BF16, name="es", tag="es")
                        nc.scalar.activation(out=es, in_=s, func=AF.Exp, scale=SCALE)
                        es_tiles[jt] = es
                        # keep PE one S-tile ahead of the PV consumer
                        if jt > 0:
                            emit_pv(jt - 1)
                    emit_pv(JT - 1)

                    # normalize: aoT = pv[0:64] / pv[64]
                    rl = nrm.tile([1, IHW], F32, name="rl", tag="rl")
                    nc.vector.reciprocal(out=rl, in_=pv[DH : DH + 1, :])
                    rlb = nrm.tile([DH, IHW], F32, name="rlb", tag="rlb")
                    nc.gpsimd.partition_broadcast(rlb, rl)
                    nc.vector.tensor_mul(out=aoT[h][:, isl], in0=pv[0:DH, :], in1=rlb)

        # ---------------- AoA output + gate ----------------
        with (
            tc.tile_pool(name="ig_ps", bufs=2, space="PSUM") as igp,
            tc.tile_pool(name="fin_p", bufs=2) as fin,
        ):
            for h in range(HPC):
                ig = igp.tile([128, N], F32, name="ig", tag="ig")
                for c in range(NCH):
                    cs = slice(c * 512, (c + 1) * 512)
                    nc.tensor.matmul(
                        ig[:, cs],
                        lhsT=(wcq_sb),
                        rhs=(qT[h][:, cs]),
                        start=True,
                        stop=False,
                    )
                    nc.tensor.matmul(
                        ig[:, cs],
                        lhsT=(wca_sb),
                        rhs=(aoT[h][:, cs]),
                        start=False,
                        stop=True,
                    )
                # sigmoid(G + bg) = 0.5 + 0.5*tanh((G + bg)/2); tanh shares the
                # exp ACT table set so no table switch. bias_sb[64:] = bg/2.
                tg = fin.tile([DH, N], F32, name="tg", tag="tg")
                nc.scalar.activation(
                    out=tg,
                    in_=ig[DH:128, :],
                    func=AF.Tanh,
                    scale=0.5,
                    bias=bias_sb[DH : 2 * DH, :],
                )
                sg = fin.tile([DH, N], F32, name="sg", tag="sg")
                nc.vector.tensor_scalar(
                    out=sg,
                    in0=tg,
                    scalar1=0.5,
                    scalar2=0.5,
                    op0=ALU.mult,
                    op1=ALU.add,
                )
                ot = fin.tile([DH, N], F32, name="ot", tag="ot")
                nc.vector.scalar_tensor_tensor(
                    out=ot,
                    in0=ig[0:DH, :],
                    scalar=bias_sb[0:DH, :],
                    in1=sg,
                    op0=ALU.add,
                    op1=ALU.mult,
                )
                nc.sync.dma_start(out=outT_d[h * DH : (h + 1) * DH, :], in_=ot)

    nc.compile()
    return nc


_NC_CACHE = None


def _get_nc():
    global _NC_CACHE
    if _NC_CACHE is None:
        _NC_CACHE = build_nc()
    return _NC_CACHE


def make_in_maps(x, Wq, Wkv, Wq_out, Wattn_out, out_bias, Wq_gate, Wattn_gate,
                 gate_bias):
    import ml_dtypes

    bf16 = ml_dtypes.bfloat16
    wcq = np.ascontiguousarray(np.concatenate([Wq_out.T, Wq_gate.T], axis=1),
                               dtype=bf16)
    wca = np.ascontiguousarray(
        np.concatenate([Wattn_out.T, Wattn_gate.T], axis=1), dtype=bf16
    )
    biases = np.concatenate(
        [out_bias.reshape(-1), 0.5 * gate_bias.reshape(-1)]
    ).astype(np.float32).reshape(2 * DH, 1)
    biases = np.ascontiguousarray(biases)
    Wk = Wkv[:, : H * DH]
    Wv = Wkv[:, H * DH :]
    xT = [np.ascontiguousarray(x[b].T).astype(bf16) for b in range(B)]
    in_maps = []
    for c in range(8):
        b, hg = c // 4, c % 4
        cols = slice(hg * INC, (hg + 1) * INC)
        in_maps.append(
            {
                "xT": xT[b],
                "wq": np.ascontiguousarray(Wq[:, cols]).astype(bf16),
                "wk": np.ascontiguousarray(Wk[:, cols]).astype(bf16),
                "wv": np.ascontiguousarray(Wv[:, cols]).astype(bf16),
                "wcq": wcq,
                "wca": wca,
                "biases": biases,
            }
        )
    return in_maps


def assemble_output(results):
    out = np.empty((B, N, H * DH), dtype=np.float32)
    for c in range(8):
        b, hg = c // 4, c % 4
        out[b, :, hg * INC : (hg + 1) * INC] = results[c]["outT"].T
    return out


def kernel(**inputs):
    from concourse.bass_utils import run_bass_kernel_spmd

    inputs = {k: np.asarray(v, dtype=np.float32) for k, v in inputs.items()}
    nc = _get_nc()
    in_maps = make_in_maps(**inputs)
    res = run_bass_kernel_spmd(nc, in_maps, core_ids=list(range(8)))
    return assemble_output(res.results)


# revision 16
# speedup vs baseline: 1.7047x; 1.0086x over previous
"""AttentionOnAttention Trainium2 kernel (8 NeuronCores, SPMD).

Sharding: core c handles batch b = c//4 and heads [4*(c%4), 4*(c%4)+4).
Each core computes its disjoint output slice out[b, :, 256*(c%4):256*(c%4+1)];
no collectives are needed.

Per-core dataflow (everything in "transposed" orientation, partition = feature):
  xT (1024, 2048) --PE--> qT_h, kT_h (64, 2048) per head, v (2048, 256) natural
  S^T[j,i] = k_h qT_h          (K=64 contraction, PSUM [128j, 1024i] tiles)
  expS^T = exp(S^T * 1/8)      (ScalarE, scale fused into ACT)
  [ao^T; L] = [v|1]^T expS^T   (K=128j accumulation, ones column gives the
                                softmax denominator for free)
  ao^T *= 1/L                  (DVE recip + GPSIMD partition broadcast + DVE mul)
  [I^T; G^T] = [Wq_out^T|Wq_gate^T]^T qT + [Wattn_out^T|Wattn_gate^T]^T ao^T
  out^T = (I^T + b) * 1/(1 + exp(-(G^T + bg)))   (sigmoid via exp: no ACT
                                                  table-set switch)
Host transposes the per-core (256, 2048) outputs back.
"""

import numpy as np
from contextlib import ExitStack

import concourse.bass as bass
import concourse.bacc as bacc
import concourse.tile as tile
from concourse import mybir

B, N, DIM, H, DH = 2, 2048, 1024, 16, 64
HPC = H // 4          # 4 heads per core
INC = HPC * DH        # 256 per-core inner width
KT = DIM // 128       # 8 contraction tiles
NCH = N // 512        # 4 free-dim chunks of 512
SCALE = float(DH) ** -0.5
F32 = mybir.dt.float32
F32R = mybir.dt.float32r
BF16 = mybir.dt.bfloat16
AF = mybir.ActivationFunctionType
ALU = mybir.AluOpType

IH = 2                # i-halves per head
IHW = N // IH         # 1024 wide i-half
JT = N // 128         # 16 j tiles


def build_nc():
    nc = bacc.Bacc(
        "TRN2",
        target_bir_lowering=False,
        debug=False,
        enable_asserts=False,
        num_devices=8,
    )
    xT_d = nc.dram_tensor("xT", (DIM, N), BF16, kind="ExternalInput").ap()
    wq_d = nc.dram_tensor("wq", (DIM, INC), BF16, kind="ExternalInput").ap()
    wk_d = nc.dram_tensor("wk", (DIM, INC), BF16, kind="ExternalInput").ap()
    wv_d = nc.dram_tensor("wv", (DIM, INC), BF16, kind="ExternalInput").ap()
    wcq_d = nc.dram_tensor("wcq", (DH, 2 * DH), BF16, kind="ExternalInput").ap()
    wca_d = nc.dram_tensor("wca", (DH, 2 * DH), BF16, kind="ExternalInput").ap()
    bias_d = nc.dram_tensor("biases", (2 * DH, 1), F32, kind="ExternalInput").ap()
    outT_d = nc.dram_tensor("outT", (INC, N), F32, kind="ExternalOutput").ap()

    with tile.TileContext(nc) as tc, ExitStack() as ctx:
        consts = ctx.enter_context(tc.tile_pool(name="consts", bufs=1))

        wcq_sb = consts.tile([DH, 2 * DH], BF16, name="wcq_sb")
        nc.sync.dma_start(out=wcq_sb, in_=wcq_d)
        wca_sb = consts.tile([DH, 2 * DH], BF16, name="wca_sb")
        nc.sync.dma_start(out=wca_sb, in_=wca_d)
        bias_sb = consts.tile([2 * DH, 1], F32, name="bias_sb")
        nc.sync.dma_start(out=bias_sb, in_=bias_d)

        # persistent per-head tensors
        qT = [consts.tile([DH, N], BF16, name=f"qT{h}") for h in range(HPC)]
        kT = [consts.tile([DH, N], BF16, name=f"kT{h}") for h in range(HPC)]
        v_aug = consts.tile([128, JT, HPC, DH + 1], BF16, name="v_aug")
        nc.vector.memset(v_aug[:, :, :, DH : DH + 1], 1.0)

        # Prefetch the exp/tanh ACT table set during the DMA prologue so the
        # first attention exp doesn't stall PE long enough to re-throttle HAM.
        warm_sb = consts.tile([128, 1], F32, name="warm_sb")
        nc.scalar.activation(out=warm_sb, in_=bias_sb, func=AF.Exp)
        nc.scalar.activation(out=warm_sb, in_=warm_sb, func=AF.Tanh)

        # ---------------- projections ----------------
        with (
            tc.tile_pool(name="xw", bufs=1) as xw,
            tc.tile_pool(name="proj_ps", bufs=2, space="PSUM") as pps,
        ):
            wq_sb = xw.tile([128, KT, INC], BF16, name="wq_sb")
            wk_sb = xw.tile([128, KT, INC], BF16, name="wk_sb")
            wv_sb = xw.tile([128, KT, INC], BF16, name="wv_sb")
            for k in range(KT):
                ks = slice(k * 128, (k + 1) * 128)
                nc.sync.dma_start(out=wq_sb[:, k, :], in_=wq_d[ks, :])
                nc.sync.dma_start(out=wk_sb[:, k, :], in_=wk_d[ks, :])
                nc.sync.dma_start(out=wv_sb[:, k, :], in_=wv_d[ks, :])

            xt_sb = xw.tile([128, KT, N], BF16, name="xt_sb")
            for c in range(NCH):
                cs = slice(c * 512, (c + 1) * 512)
                for k in range(KT):
                    nc.sync.dma_start(
                        out=xt_sb[:, k, cs], in_=xT_d[k * 128 : (k + 1) * 128, cs]
                    )

            for c in range(NCH):
                cs = slice(c * 512, (c + 1) * 512)
                for m in range(2):  # inner m-tile: heads 2m, 2m+1
                    for wsb, dst in ((wq_sb, qT), (wk_sb, kT)):
                        ps = pps.tile([128, 512], F32, name="ps_qk", tag="ps_qk")
                        for k in range(KT):
                            nc.tensor.matmul(
                                ps,
                                lhsT=(wsb[:, k, m * 128 : (m + 1) * 128]),
                                rhs=(xt_sb[:, k, cs]),
                                start=(k == 0),
                                stop=(k == KT - 1),
                            )
                        nc.vector.tensor_copy(out=dst[2 * m][:, cs], in_=ps[0:DH, :])
                        nc.vector.tensor_copy(
                            out=dst[2 * m + 1][:, cs], in_=ps[DH:128, :]
                        )
                # v natural: i-tiles of this chunk
                for it in range(c * 4, c * 4 + 4):
                    psv = pps.tile([128, INC], F32, name="ps_v", tag="ps_v")
                    for k in range(KT):
                        nc.tensor.matmul(
                            psv,
                            lhsT=(xt_sb[:, k, it * 128 : (it + 1) * 128]),
                            rhs=(wv_sb[:, k, :]),
                            start=(k == 0),
                            stop=(k == KT - 1),
                        )
                    nc.vector.tensor_copy(
                        out=v_aug[:, it, :, 0:DH],
                        in_=psv.rearrange("p (h d) -> p h d", h=HPC),
                    )

        # ---------------- attention ----------------
        aoT = [consts.tile([DH, N], BF16, name=f"aoT{h}") for h in range(HPC)]
        aoU = [consts.tile([DH + 1, N], F32, name=f"aoU{h}") for h in range(HPC)]
        with (
            tc.tile_pool(name="s_ps", bufs=2, space="PSUM") as sps,
            tc.tile_pool(name="pv_ps", bufs=2, space="PSUM") as pvps,
            tc.tile_pool(name="es_p", bufs=4) as esp,
            tc.tile_pool(name="norm_p", bufs=2) as nrm,
        ):
            for h in range(HPC):
                for ih in range(IH):
                    isl = slice(ih * IHW, (ih + 1) * IHW)
                    pv = pvps.tile([DH + 1, IHW], F32, name="pv", tag="pv")
                    es_tiles = [None] * JT

                    def emit_pv(jt):
                        for cc in range(IHW // 512):
                            nc.tensor.matmul(
                                pv[:, cc * 512 : (cc + 1) * 512],
                                lhsT=(v_aug[:, jt, h, :]),
                                rhs=(es_tiles[jt][:, cc * 512 : (cc + 1) * 512]),
                                start=(jt == 0),
                                stop=(jt == JT - 1),
                            )

                    for jt in range(JT):
                        s = sps.tile([128, IHW], F32, name="s", tag="s")
                        for cc in range(IHW // 512):
                            qs = slice(ih * IHW + cc * 512, ih * IHW + (cc + 1) * 512)
                            nc.tensor.matmul(
                                s[:, cc * 512 : (cc + 1) * 512],
                                lhsT=(kT[h][:, jt * 128 : (jt + 1) * 128]),
                                rhs=(qT[h][:, qs]),
                                start=True,
                                stop=True,
                            )
                        es = esp.tile([128, IHW], BF16, name="es", tag="es")
                        nc.scalar.activation(out=es, in_=s, func=AF.Exp, scale=SCALE)
                        es_tiles[jt] = es
                        # keep PE one S-tile ahead of the PV consumer
                        if jt > 0:
                            emit_pv(jt - 1)
                    emit_pv(JT - 1)

                    # evacuate PSUM with one fast copy; the slow reciprocal +
                    # broadcast + normalize run lazily on DVE/GPSIMD while
                    # PE/ACT proceed with the next head
                    nc.vector.tensor_copy(out=aoU[h][:, isl], in_=pv)
                    rl = nrm.tile([1, IHW], F32, name="rl", tag="rl")
                    nc.vector.reciprocal(out=rl, in_=aoU[h][DH : DH + 1, isl])
                    rlb = nrm.tile([DH, IHW], F32, name="rlb", tag="rlb")
                    nc.gpsimd.partition_broadcast(rlb, rl)
                    nc.vector.tensor_mul(
                        out=aoT[h][:, isl], in0=aoU[h][0:DH, isl], in1=rlb
                    )

        # ---------------- AoA output + gate ----------------
        with (
            tc.tile_pool(name="ig_ps", bufs=2, space="PSUM") as igp,
            tc.tile_pool(name="fin_p", bufs=2) as fin,
        ):
            for h in range(HPC):
                ig = igp.tile([128, N], F32, name="ig", tag="ig")
                for c in range(NCH):
                    cs = slice(c * 512, (c + 1) * 512)
                    nc.tensor.matmul(
                        ig[:, cs],
                        lhsT=(wcq_sb),
                        rhs=(qT[h][:, cs]),
                        start=True,
                        stop=False,
                    )
                    nc.tensor.matmul(
                        ig[:, cs],
                        lhsT=(wca_sb),
                        rhs=(aoT[h][:, cs]),
                        start=False,
                        stop=True,
                    )
                # sigmoid(G + bg) = 0.5 + 0.5*tanh((G + bg)/2); tanh shares the
                # exp ACT table set so no table switch. bias_sb[64:] = bg/2.
                tg = fin.tile([DH, N], F32, name="tg", tag="tg")
                nc.scalar.activation(
                    out=tg,
                    in_=ig[DH:128, :],
                    func=AF.Tanh,
                    scale=0.5,
                    bias=bias_sb[DH : 2 * DH, :],
                )
                sg = fin.tile([DH, N], F32, name="sg", tag="sg")
                nc.vector.tensor_scalar(
                    out=sg,
                    in0=tg,
                    scalar1=0.5,
                    scalar2=0.5,
                    op0=ALU.mult,
                    op1=ALU.add,
                )
                ot = fin.tile([DH, N], F32, name="ot", tag="ot")
                nc.vector.scalar_tensor_tensor(
                    out=ot,
                    in0=ig[0:DH, :],
                    scalar=bias_sb[0:DH, :],
                    in1=sg,
                    op0=ALU.add,
                    op1=ALU.mult,
                )
                nc.sync.dma_start(out=outT_d[h * DH : (h + 1) * DH, :], in_=ot)

    nc.compile()
    return nc


_NC_CACHE = None


def _get_nc():
    global _NC_CACHE
    if _NC_CACHE is None:
        _NC_CACHE = build_nc()
    return _NC_CACHE


def make_in_maps(x, Wq, Wkv, Wq_out, Wattn_out, out_bias, Wq_gate, Wattn_gate,
                 gate_bias):
    import ml_dtypes

    bf16 = ml_dtypes.bfloat16
    wcq = np.ascontiguousarray(np.concatenate([Wq_out.T, Wq_gate.T], axis=1),
                               dtype=bf16)
    wca = np.ascontiguousarray(
        np.concatenate([Wattn_out.T, Wattn_gate.T], axis=1), dtype=bf16
    )
    biases = np.concatenate(
        [out_bias.reshape(-1), 0.5 * gate_bias.reshape(-1)]
    ).astype(np.float32).reshape(2 * DH, 1)
    biases = np.ascontiguousarray(biases)
    Wk = Wkv[:, : H * DH]
    Wv = Wkv[:, H * DH :]
    xT = [np.ascontiguousarray(x[b].T).astype(bf16) for b in range(B)]
    in_maps = []
    for c in range(8):
        b, hg = c // 4, c % 4
        cols = slice(hg * INC, (hg + 1) * INC)
        in_maps.append(
            {
                "xT": xT[b],
                "wq": np.ascontiguousarray(Wq[:, cols]).astype(bf16),
                "wk": np.ascontiguousarray(Wk[:, cols]).astype(bf16),
                "wv": np.ascontiguousarray(Wv[:, cols]).astype(bf16),
                "wcq": wcq,
                "wca": wca,
                "biases": biases,
            }
        )
    return in_maps


def assemble_output(results):
    out = np.empty((B, N, H * DH), dtype=np.float32)
    for c in range(8):
        b, hg = c // 4, c % 4
        out[b, :, hg * INC : (hg + 1) * INC] = results[c]["outT"].T
    return out


def kernel(**inputs):
    from concourse.bass_utils import run_bass_kernel_spmd

    inputs = {k: np.asarray(v, dtype=np.float32) for k, v in inputs.items()}
    nc = _get_nc()
    in_maps = make_in_maps(**inputs)
    res = run_bass_kernel_spmd(nc, in_maps, core_ids=list(range(8)))
    return assemble_output(res.results)


# revision 19
# speedup vs baseline: 1.7426x; 1.0222x over previous
"""AttentionOnAttention Trainium2 kernel (8 NeuronCores, SPMD).

Sharding: core c handles batch b = c//4 and heads [4*(c%4), 4*(c%4)+4).
Each core computes its disjoint output slice out[b, :, 256*(c%4):256*(c%4+1)];
no collectives are needed.

Per-core dataflow (everything in "transposed" orientation, partition = feature):
  xT (1024, 2048) --PE--> qT_h, kT_h (64, 2048) per head, v (2048, 256) natural
  S^T[j,i] = k_h qT_h          (K=64 contraction, PSUM [128j, 1024i] tiles)
  expS^T = exp(S^T * 1/8)      (ScalarE, scale fused into ACT)
  [ao^T; L] = [v|1]^T expS^T   (K=128j accumulation, ones column gives the
                                softmax denominator for free)
  ao^T *= 1/L                  (DVE recip + GPSIMD partition broadcast + DVE mul)
  [I^T; G^T] = [Wq_out^T|Wq_gate^T]^T qT + [Wattn_out^T|Wattn_gate^T]^T ao^T
  out^T = (I^T + b) * 1/(1 + exp(-(G^T + bg)))   (sigmoid via exp: no ACT
                                                  table-set switch)
Host transposes the per-core (256, 2048) outputs back.
"""

import numpy as np
from contextlib import ExitStack

import concourse.bass as bass
import concourse.bacc as bacc
import concourse.tile as tile
from concourse import mybir

B, N, DIM, H, DH = 2, 2048, 1024, 16, 64
HPC = H // 4          # 4 heads per core
INC = HPC * DH        # 256 per-core inner width
KT = DIM // 128       # 8 contraction tiles
NCH = N // 512        # 4 free-dim chunks of 512
SCALE = float(DH) ** -0.5
F32 = mybir.dt.float32
F32R = mybir.dt.float32r
BF16 = mybir.dt.bfloat16
AF = mybir.ActivationFunctionType
ALU = mybir.AluOpType

IH = 2                # i-halves per head
IHW = N // IH         # 1024 wide i-half
JT = N // 128         # 16 j tiles


def build_nc():
    nc = bacc.Bacc(
        "TRN2",
        target_bir_lowering=False,
        debug=False,
        enable_asserts=False,
        num_devices=8,
    )
    xT_d = nc.dram_tensor("xT", (KT, 128, N), BF16, kind="ExternalInput").ap()
    wq_d = nc.dram_tensor("wq", (DIM, INC), BF16, kind="ExternalInput").ap()
    wk_d = nc.dram_tensor("wk", (DIM, INC), BF16, kind="ExternalInput").ap()
    wv_d = nc.dram_tensor("wv", (DIM, INC), BF16, kind="ExternalInput").ap()
    wcq_d = nc.dram_tensor("wcq", (DH, 2 * DH), BF16, kind="ExternalInput").ap()
    wca_d = nc.dram_tensor("wca", (DH, 2 * DH), BF16, kind="ExternalInput").ap()
    bias_d = nc.dram_tensor("biases", (2 * DH, 1), F32, kind="ExternalInput").ap()
    outT_d = nc.dram_tensor("outT", (INC, N), F32, kind="ExternalOutput").ap()

    with tile.TileContext(nc) as tc, ExitStack() as ctx:
        consts = ctx.enter_context(tc.tile_pool(name="consts", bufs=1))

        wcq_sb = consts.tile([DH, 2 * DH], BF16, name="wcq_sb")
        nc.sync.dma_start(out=wcq_sb, in_=wcq_d)
        wca_sb = consts.tile([DH, 2 * DH], BF16, name="wca_sb")
        nc.sync.dma_start(out=wca_sb, in_=wca_d)
        bias_sb = consts.tile([2 * DH, 1], F32, name="bias_sb")
        nc.sync.dma_start(out=bias_sb, in_=bias_d)

        # persistent per-head tensors
        qT = [consts.tile([DH, N], BF16, name=f"qT{h}") for h in range(HPC)]
        kT = [consts.tile([DH, N], BF16, name=f"kT{h}") for h in range(HPC)]
        v_aug = consts.tile([128, JT, HPC, DH + 1], BF16, name="v_aug")
        nc.vector.memset(v_aug[:, :, :, DH : DH + 1], 1.0)

        # Prefetch the exp/tanh ACT table set during the DMA prologue so the
        # first attention exp doesn't stall PE long enough to re-throttle HAM.
        warm_sb = consts.tile([128, 1], F32, name="warm_sb")
        nc.scalar.activation(out=warm_sb, in_=bias_sb, func=AF.Exp)
        nc.scalar.activation(out=warm_sb, in_=warm_sb, func=AF.Tanh)

        # ---------------- projections ----------------
        with (
            tc.tile_pool(name="xw", bufs=1) as xw,
            tc.tile_pool(name="proj_ps", bufs=2, space="PSUM") as pps,
        ):
            # DMA order matters: the first projection (q of chunk 0) is gated
            # on wq + wk + xt chunk 0, so those go first.
            wq_sb = xw.tile([128, KT, INC], BF16, name="wq_sb")
            wk_sb = xw.tile([128, KT, INC], BF16, name="wk_sb")
            wv_sb = xw.tile([128, KT, INC], BF16, name="wv_sb")
            xt_sb = xw.tile([128, KT, N], BF16, name="xt_sb")
            for k in range(KT):
                ks = slice(k * 128, (k + 1) * 128)
                nc.sync.dma_start(out=wq_sb[:, k, :], in_=wq_d[ks, :])
                nc.sync.dma_start(out=wk_sb[:, k, :], in_=wk_d[ks, :])
            for k in range(KT):
                nc.sync.dma_start(out=xt_sb[:, k, 0:512], in_=xT_d[k, :, 0:512])
            for k in range(KT):
                ks = slice(k * 128, (k + 1) * 128)
                nc.sync.dma_start(out=wv_sb[:, k, :], in_=wv_d[ks, :])
            for c in range(1, NCH):
                cs = slice(c * 512, (c + 1) * 512)
                for k in range(KT):
                    nc.sync.dma_start(out=xt_sb[:, k, cs], in_=xT_d[k, :, cs])

            for c in range(NCH):
                cs = slice(c * 512, (c + 1) * 512)
                for m in range(2):  # inner m-tile: heads 2m, 2m+1
                    for wsb, dst in ((wq_sb, qT), (wk_sb, kT)):
                        ps = pps.tile([128, 512], F32, name="ps_qk", tag="ps_qk")
                        for k in range(KT):
                            nc.tensor.matmul(
                                ps,
                                lhsT=(wsb[:, k, m * 128 : (m + 1) * 128]),
                                rhs=(xt_sb[:, k, cs]),
                                start=(k == 0),
                                stop=(k == KT - 1),
                            )
                        nc.vector.tensor_copy(out=dst[2 * m][:, cs], in_=ps[0:DH, :])
                        nc.vector.tensor_copy(
                            out=dst[2 * m + 1][:, cs], in_=ps[DH:128, :]
                        )
                # v natural: i-tiles of this chunk
                for it in range(c * 4, c * 4 + 4):
                    psv = pps.tile([128, INC], F32, name="ps_v", tag="ps_v")
                    for k in range(KT):
                        nc.tensor.matmul(
                            psv,
                            lhsT=(xt_sb[:, k, it * 128 : (it + 1) * 128]),
                            rhs=(wv_sb[:, k, :]),
                            start=(k == 0),
                            stop=(k == KT - 1),
                        )
                    nc.vector.tensor_copy(
                        out=v_aug[:, it, :, 0:DH],
                        in_=psv.rearrange("p (h d) -> p h d", h=HPC),
                    )

        # ---------------- attention ----------------
        aoT = [consts.tile([DH, N], BF16, name=f"aoT{h}") for h in range(HPC)]
        aoU = [consts.tile([DH + 1, N], F32, name=f"aoU{h}") for h in range(HPC)]
        with (
            tc.tile_pool(name="s_ps", bufs=2, space="PSUM") as sps,
            tc.tile_pool(name="pv_ps", bufs=2, space="PSUM") as pvps,
            tc.tile_pool(name="es_p", bufs=4) as esp,
            tc.tile_pool(name="norm_p", bufs=2) as nrm,
        ):
            for h in range(HPC):
                for ih in range(IH):
                    isl = slice(ih * IHW, (ih + 1) * IHW)
                    pv = pvps.tile([DH + 1, IHW], F32, name="pv", tag="pv")
                    es_tiles = [None] * JT

                    def emit_pv(jt):
                        for cc in range(IHW // 512):
                            nc.tensor.matmul(
                                pv[:, cc * 512 : (cc + 1) * 512],
                                lhsT=(v_aug[:, jt, h, :]),
                                rhs=(es_tiles[jt][:, cc * 512 : (cc + 1) * 512]),
                                start=(jt == 0),
                                stop=(jt == JT - 1),
                            )

                    for jt in range(JT):
                        s = sps.tile([128, IHW], F32, name="s", tag="s")
                        for cc in range(IHW // 512):
                            qs = slice(ih * IHW + cc * 512, ih * IHW + (cc + 1) * 512)
                            nc.tensor.matmul(
                                s[:, cc * 512 : (cc + 1) * 512],
                                lhsT=(kT[h][:, jt * 128 : (jt + 1) * 128]),
                                rhs=(qT[h][:, qs]),
                                start=True,
                                stop=True,
                            )
                        es = esp.tile([128, IHW], BF16, name="es", tag="es")
                        nc.scalar.activation(out=es, in_=s, func=AF.Exp, scale=SCALE)
                        es_tiles[jt] = es
                        # keep PE one S-tile ahead of the PV consumer
                        if jt > 0:
                            emit_pv(jt - 1)
                    emit_pv(JT - 1)

                    # evacuate PSUM with one fast copy; the slow reciprocal +
                    # broadcast + normalize run lazily on DVE/GPSIMD while
                    # PE/ACT proceed with the next head
                    nc.vector.tensor_copy(out=aoU[h][:, isl], in_=pv)
                    rl = nrm.tile([1, IHW], F32, name="rl", tag="rl")
                    nc.vector.reciprocal(out=rl, in_=aoU[h][DH : DH + 1, isl])
                    rlb = nrm.tile([DH, IHW], F32, name="rlb", tag="rlb")
                    nc.gpsimd.partition_broadcast(rlb, rl)
                    nc.vector.tensor_mul(
                        out=aoT[h][:, isl], in0=aoU[h][0:DH, isl], in1=rlb
                    )

        # ---------------- AoA output + gate ----------------
        with (
            tc.tile_pool(name="ig_ps", bufs=2, space="PSUM") as igp,
            tc.tile_pool(name="fin_p", bufs=2) as fin,
        ):
            for h in range(HPC):
                ig = igp.tile([128, N], F32, name="ig", tag="ig")
                for c in range(NCH):
                    cs = slice(c * 512, (c + 1) * 512)
                    nc.tensor.matmul(
                        ig[:, cs],
                        lhsT=(wcq_sb),
                        rhs=(qT[h][:, cs]),
                        start=True,
                        stop=False,
                    )
                    nc.tensor.matmul(
                        ig[:, cs],
                        lhsT=(wca_sb),
                        rhs=(aoT[h][:, cs]),
                        start=False,
                        stop=True,
                    )
                # sigmoid(G + bg) = 0.5 + 0.5*tanh((G + bg)/2); tanh shares the
                # exp ACT table set so no table switch. bias_sb[64:] = bg/2.
                tg = fin.tile([DH, N], F32, name="tg", tag="tg")
                nc.scalar.activation(
                    out=tg,
                    in_=ig[DH:128, :],
                    func=AF.Tanh,
                    scale=0.5,
                    bias=bias_sb[DH : 2 * DH, :],
                )
                sg = fin.tile([DH, N], F32, name="sg", tag="sg")
                nc.vector.tensor_scalar(
                    out=sg,
                    in0=tg,
                    scalar1=0.5,
                    scalar2=0.5,
                    op0=ALU.mult,
                    op1=ALU.add,
                )
                ot = fin.tile([DH, N], F32, name="ot", tag="ot")
                nc.vector.scalar_tensor_tensor(
                    out=ot,
                    in0=ig[0:DH, :],
                    scalar=bias_sb[0:DH, :],
                    in1=sg,
                    op0=ALU.add,
                    op1=ALU.mult,
                )
                nc.sync.dma_start(out=outT_d[h * DH : (h + 1) * DH, :], in_=ot)

    nc.compile()
    return nc


_NC_CACHE = None


def _get_nc():
    global _NC_CACHE
    if _NC_CACHE is None:
        _NC_CACHE = build_nc()
    return _NC_CACHE


def make_in_maps(x, Wq, Wkv, Wq_out, Wattn_out, out_bias, Wq_gate, Wattn_gate,
                 gate_bias):
    import ml_dtypes

    bf16 = ml_dtypes.bfloat16
    wcq = np.ascontiguousarray(np.concatenate([Wq_out.T, Wq_gate.T], axis=1),
                               dtype=bf16)
    wca = np.ascontiguousarray(
        np.concatenate([Wattn_out.T, Wattn_gate.T], axis=1), dtype=bf16
    )
    biases = np.concatenate(
        [out_bias.reshape(-1), 0.5 * gate_bias.reshape(-1)]
    ).astype(np.float32).reshape(2 * DH, 1)
    biases = np.ascontiguousarray(biases)
    Wk = Wkv[:, : H * DH]
    Wv = Wkv[:, H * DH :]
    xT = [
        np.ascontiguousarray(
            x[b].T.reshape(KT, 128, N)
        ).astype(bf16)
        for b in range(B)
    ]
    in_maps = []
    for c in range(8):
        b, hg = c // 4, c % 4
        cols = slice(hg * INC, (hg + 1) * INC)
        in_maps.append(
            {
                "xT": xT[b],
                "wq": np.ascontiguousarray(Wq[:, cols]).astype(bf16),
                "wk": np.ascontiguousarray(Wk[:, cols]).astype(bf16),
                "wv": np.ascontiguousarray(Wv[:, cols]).astype(bf16),
                "wcq": wcq,
                "wca": wca,
                "biases": biases,
            }
        )
    return in_maps


def assemble_output(results):
    out = np.empty((B, N, H * DH), dtype=np.float32)
    for c in range(8):
        b, hg = c // 4, c % 4
        out[b, :, hg * INC : (hg + 1) * INC] = results[c]["outT"].T
    return out


def kernel(**inputs):
    from concourse.bass_utils import run_bass_kernel_spmd

    inputs = {k: np.asarray(v, dtype=np.float32) for k, v in inputs.items()}
    nc = _get_nc()
    in_maps = make_in_maps(**inputs)
    res = run_bass_kernel_spmd(nc, in_maps, core_ids=list(range(8)))
    return assemble_output(res.results)


# revision 21
# speedup vs baseline: 1.8117x; 1.0396x over previous
"""AttentionOnAttention Trainium2 kernel (8 NeuronCores, SPMD).

Sharding: core c handles batch b = c//4 and heads [4*(c%4), 4*(c%4)+4).
Each core computes its disjoint output slice out[b, :, 256*(c%4):256*(c%4+1)];
no collectives are needed.

Per-core dataflow (everything in "transposed" orientation, partition = feature):
  xT (1024, 2048) --PE--> qT_h, kT_h (64, 2048) per head, v (2048, 256) natural
  S^T[j,i] = k_h qT_h          (K=64 contraction, PSUM [128j, 1024i] tiles)
  expS^T = exp(S^T * 1/8)      (ScalarE, scale fused into ACT)
  [ao^T; L] = [v|1]^T expS^T   (K=128j accumulation, ones column gives the
                                softmax denominator for free)
  ao^T *= 1/L                  (DVE recip + GPSIMD partition broadcast + DVE mul)
  [I^T; G^T] = [Wq_out^T|Wq_gate^T]^T qT + [Wattn_out^T|Wattn_gate^T]^T ao^T
  out^T = (I^T + b) * 1/(1 + exp(-(G^T + bg)))   (sigmoid via exp: no ACT
                                                  table-set switch)
Host transposes the per-core (256, 2048) outputs back.
"""

import numpy as np
from contextlib import ExitStack

import concourse.bass as bass
import concourse.bacc as bacc
import concourse.tile as tile
from concourse import mybir

B, N, DIM, H, DH = 2, 2048, 1024, 16, 64
HPC = H // 4          # 4 heads per core
INC = HPC * DH        # 256 per-core inner width
KT = DIM // 128       # 8 contraction tiles
NCH = N // 512        # 4 free-dim chunks of 512
SCALE = float(DH) ** -0.5
F32 = mybir.dt.float32
F32R = mybir.dt.float32r
BF16 = mybir.dt.bfloat16
AF = mybir.ActivationFunctionType
ALU = mybir.AluOpType

IH = 2                # i-halves per head
IHW = N // IH         # 1024 wide i-half
JT = N // 128         # 16 j tiles


def build_nc():
    nc = bacc.Bacc(
        "TRN2",
        target_bir_lowering=False,
        debug=False,
        enable_asserts=False,
        num_devices=8,
    )
    xT_d = nc.dram_tensor("xT", (KT, 128, N), BF16, kind="ExternalInput").ap()
    wq_d = nc.dram_tensor("wq", (DIM, INC), BF16, kind="ExternalInput").ap()
    wk_d = nc.dram_tensor("wk", (DIM, INC), BF16, kind="ExternalInput").ap()
    wv_d = nc.dram_tensor("wv", (DIM, INC), BF16, kind="ExternalInput").ap()
    wcq_d = nc.dram_tensor("wcq", (DH, 2 * DH), BF16, kind="ExternalInput").ap()
    wca_d = nc.dram_tensor("wca", (DH, 2 * DH), BF16, kind="ExternalInput").ap()
    bias_d = nc.dram_tensor("biases", (2 * DH, 1), F32, kind="ExternalInput").ap()
    outT_d = nc.dram_tensor("outT", (INC, N), F32, kind="ExternalOutput").ap()

    with tile.TileContext(nc) as tc, ExitStack() as ctx:
        consts = ctx.enter_context(tc.tile_pool(name="consts", bufs=1))

        wcq_sb = consts.tile([DH, 2 * DH], BF16, name="wcq_sb")
        nc.sync.dma_start(out=wcq_sb, in_=wcq_d)
        wca_sb = consts.tile([DH, 2 * DH], BF16, name="wca_sb")
        nc.sync.dma_start(out=wca_sb, in_=wca_d)
        bias_sb = consts.tile([2 * DH, 1], F32, name="bias_sb")
        nc.sync.dma_start(out=bias_sb, in_=bias_d)

        # persistent per-head tensors
        qT = [consts.tile([DH, N], BF16, name=f"qT{h}") for h in range(HPC)]
        kT = [consts.tile([DH, N], BF16, name=f"kT{h}") for h in range(HPC)]
        v_aug = consts.tile([128, JT, HPC, DH + 1], BF16, name="v_aug")
        nc.vector.memset(v_aug[:, :, :, DH : DH + 1], 1.0)

        # Prefetch the exp/tanh ACT table set during the DMA prologue so the
        # first attention exp doesn't stall PE long enough to re-throttle HAM.
        warm_sb = consts.tile([128, 1], F32, name="warm_sb")
        nc.scalar.activation(out=warm_sb, in_=bias_sb, func=AF.Exp)
        nc.scalar.activation(out=warm_sb, in_=warm_sb, func=AF.Tanh)

        # ---------------- projections ----------------
        with (
            tc.tile_pool(name="xw", bufs=1) as xw,
            tc.tile_pool(name="proj_ps", bufs=2, space="PSUM") as pps,
        ):
            # DMA order matters: the first projection (q of chunk 0) is gated
            # on wq + wk + xt chunk 0, so those go first.
            wq_sb = xw.tile([128, KT, INC], BF16, name="wq_sb")
            wk_sb = xw.tile([128, KT, INC], BF16, name="wk_sb")
            wv_sb = xw.tile([128, KT, INC], BF16, name="wv_sb")
            xt_sb = xw.tile([128, KT, N], BF16, name="xt_sb")
            for k in range(KT):
                ks = slice(k * 128, (k + 1) * 128)
                nc.sync.dma_start(out=wq_sb[:, k, :], in_=wq_d[ks, :])
                nc.sync.dma_start(out=wk_sb[:, k, :], in_=wk_d[ks, :])
            for k in range(KT):
                nc.sync.dma_start(out=xt_sb[:, k, 0:512], in_=xT_d[k, :, 0:512])
            for k in range(KT):
                ks = slice(k * 128, (k + 1) * 128)
                nc.sync.dma_start(out=wv_sb[:, k, :], in_=wv_d[ks, :])
            for c in range(1, NCH):
                cs = slice(c * 512, (c + 1) * 512)
                for k in range(KT):
                    nc.sync.dma_start(out=xt_sb[:, k, cs], in_=xT_d[k, :, cs])

            for c in range(NCH):
                cs = slice(c * 512, (c + 1) * 512)
                for m in range(2):  # inner m-tile: heads 2m, 2m+1
                    for wsb, dst in ((wq_sb, qT), (wk_sb, kT)):
                        ps = pps.tile([128, 512], F32, name="ps_qk", tag="ps_qk")
                        for k in range(KT):
                            nc.tensor.matmul(
                                ps,
                                lhsT=(wsb[:, k, m * 128 : (m + 1) * 128]),
                                rhs=(xt_sb[:, k, cs]),
                                start=(k == 0),
                                stop=(k == KT - 1),
                            )
                        nc.vector.tensor_copy(out=dst[2 * m][:, cs], in_=ps[0:DH, :])
                        nc.vector.tensor_copy(
                            out=dst[2 * m + 1][:, cs], in_=ps[DH:128, :]
                        )
                # v natural: i-tiles of this chunk
                for it in range(c * 4, c * 4 + 4):
                    psv = pps.tile([128, INC], F32, name="ps_v", tag="ps_v")
                    for k in range(KT):
                        nc.tensor.matmul(
                            psv,
                            lhsT=(xt_sb[:, k, it * 128 : (it + 1) * 128]),
                            rhs=(wv_sb[:, k, :]),
                            start=(k == 0),
                            stop=(k == KT - 1),
                        )
                    nc.vector.tensor_copy(
                        out=v_aug[:, it, :, 0:DH],
                        in_=psv.rearrange("p (h d) -> p h d", h=HPC),
                    )

        # ---------------- attention ----------------
        aoT = [consts.tile([DH, N], BF16, name=f"aoT{h}") for h in range(HPC)]
        aoU = [consts.tile([DH + 1, N], F32, name=f"aoU{h}") for h in range(HPC)]
        with (
            tc.tile_pool(name="s_ps", bufs=3, space="PSUM") as sps,
            tc.tile_pool(name="pv_ps", bufs=2, space="PSUM") as pvps,
            tc.tile_pool(name="es_p", bufs=4) as esp,
            tc.tile_pool(name="norm_p", bufs=2) as nrm,
        ):
            for h in range(HPC):
                for ih in range(IH):
                    isl = slice(ih * IHW, (ih + 1) * IHW)
                    pv = [
                        pvps.tile([DH + 1, 512], F32, name=f"pv{cc}", tag="pv")
                        for cc in range(IHW // 512)
                    ]
                    es_tiles = [None] * JT

                    def emit_pv(jt):
                        for cc in range(IHW // 512):
                            nc.tensor.matmul(
                                pv[cc],
                                lhsT=(v_aug[:, jt, h, :]),
                                rhs=(es_tiles[jt][:, cc * 512 : (cc + 1) * 512]),
                                start=(jt == 0),
                                stop=(jt == JT - 1),
                            )

                    for jt in range(JT):
                        s = sps.tile([128, IHW], F32, name="s", tag="s")
                        for cc in range(IHW // 512):
                            qs = slice(ih * IHW + cc * 512, ih * IHW + (cc + 1) * 512)
                            nc.tensor.matmul(
                                s[:, cc * 512 : (cc + 1) * 512],
                                lhsT=(kT[h][:, jt * 128 : (jt + 1) * 128]),
                                rhs=(qT[h][:, qs]),
                                start=True,
                                stop=True,
                            )
                        es = esp.tile([128, IHW], BF16, name="es", tag="es")
                        nc.scalar.activation(out=es, in_=s, func=AF.Exp, scale=SCALE)
                        es_tiles[jt] = es
                        # keep PE one S-tile ahead of the PV consumer
                        if jt > 0:
                            emit_pv(jt - 1)
                    emit_pv(JT - 1)

                    # evacuate PSUM with one fast copy; the slow reciprocal +
                    # broadcast + normalize run lazily on DVE/GPSIMD while
                    # PE/ACT proceed with the next head
                    for cc in range(IHW // 512):
                        ccs = slice(ih * IHW + cc * 512, ih * IHW + (cc + 1) * 512)
                        nc.vector.tensor_copy(out=aoU[h][:, ccs], in_=pv[cc])
                    rl = nrm.tile([1, IHW], F32, name="rl", tag="rl")
                    nc.vector.reciprocal(out=rl, in_=aoU[h][DH : DH + 1, isl])
                    rlb = nrm.tile([DH, IHW], F32, name="rlb", tag="rlb")
                    nc.gpsimd.partition_broadcast(rlb, rl)
                    nc.vector.tensor_mul(
                        out=aoT[h][:, isl], in0=aoU[h][0:DH, isl], in1=rlb
                    )

        # ---------------- AoA output + gate ----------------
        with (
            tc.tile_pool(name="ig_ps", bufs=2, space="PSUM") as igp,
            tc.tile_pool(name="fin_p", bufs=2) as fin,
        ):
            for h in range(HPC):
                ig = igp.tile([128, N], F32, name="ig", tag="ig")
                for c in range(NCH):
                    cs = slice(c * 512, (c + 1) * 512)
                    nc.tensor.matmul(
                        ig[:, cs],
                        lhsT=(wcq_sb),
                        rhs=(qT[h][:, cs]),
                        start=True,
                        stop=False,
                    )
                    nc.tensor.matmul(
                        ig[:, cs],
                        lhsT=(wca_sb),
                        rhs=(aoT[h][:, cs]),
                        start=False,
                        stop=True,
                    )
                # sigmoid(G + bg) = 0.5 + 0.5*tanh((G + bg)/2); tanh shares the
                # exp ACT table set so no table switch. bias_sb[64:] = bg/2.
                tg = fin.tile([DH, N], F32, name="tg", tag="tg")
                nc.scalar.activation(
                    out=tg,
                    in_=ig[DH:128, :],
                    func=AF.Tanh,
                    scale=0.5,
                    bias=bias_sb[DH : 2 * DH, :],
                )
                sg = fin.tile([DH, N], F32, name="sg", tag="sg")
                nc.vector.tensor_scalar(
                    out=sg,
                    in0=tg,
                    scalar1=0.5,
                    scalar2=0.5,
                    op0=ALU.mult,
                    op1=ALU.add,
                )
                ot = fin.tile([DH, N], F32, name="ot", tag="ot")
                nc.vector.scalar_tensor_tensor(
                    out=ot,
                    in0=ig[0:DH, :],
                    scalar=bias_sb[0:DH, :],
                    in1=sg,
                    op0=ALU.add,
                    op1=ALU.mult,
                )
                nc.sync.dma_start(out=outT_d[h * DH : (h + 1) * DH, :], in_=ot)

    nc.compile()
    return nc


_NC_CACHE = None


def _get_nc():
    global _NC_CACHE
    if _NC_CACHE is None:
        _NC_CACHE = build_nc()
    return _NC_CACHE


def make_in_maps(x, Wq, Wkv, Wq_out, Wattn_out, out_bias, Wq_gate, Wattn_gate,
                 gate_bias):
    import ml_dtypes

    bf16 = ml_dtypes.bfloat16
    wcq = np.ascontiguousarray(np.concatenate([Wq_out.T, Wq_gate.T], axis=1),
                               dtype=bf16)
    wca = np.ascontiguousarray(
        np.concatenate([Wattn_out.T, Wattn_gate.T], axis=1), dtype=bf16
    )
    biases = np.concatenate(
        [out_bias.reshape(-1), 0.5 * gate_bias.reshape(-1)]
    ).astype(np.float32).reshape(2 * DH, 1)
    biases = np.ascontiguousarray(biases)
    Wk = Wkv[:, : H * DH]
    Wv = Wkv[:, H * DH :]
    xT = [
        np.ascontiguousarray(
            x[b].T.reshape(KT, 128, N)
        ).astype(bf16)
        for b in range(B)
    ]
    in_maps = []
    for c in range(8):
        b, hg = c // 4, c % 4
        cols = slice(hg * INC, (hg + 1) * INC)
        in_maps.append(
            {
                "xT": xT[b],
                "wq": np.ascontiguousarray(Wq[:, cols]).astype(bf16),
                "wk": np.ascontiguousarray(Wk[:, cols]).astype(bf16),
                "wv": np.ascontiguousarray(Wv[:, cols]).astype(bf16),
                "wcq": wcq,
                "wca": wca,
                "biases": biases,
            }
        )
    return in_maps


def assemble_output(results):
    out = np.empty((B, N, H * DH), dtype=np.float32)
    for c in range(8):
        b, hg = c // 4, c % 4
        out[b, :, hg * INC : (hg + 1) * INC] = results[c]["outT"].T
    return out


def kernel(**inputs):
    from concourse.bass_utils import run_bass_kernel_spmd

    inputs = {k: np.asarray(v, dtype=np.float32) for k, v in inputs.items()}
    nc = _get_nc()
    in_maps = make_in_maps(**inputs)
    res = run_bass_kernel_spmd(nc, in_maps, core_ids=list(range(8)))
    return assemble_output(res.results)
